# revision 52
# baseline (speedup 1.0000x reference)
"""Trainium2 Bass kernel for nn_MACTitanLayer (MAC Titan layer, 8 cores).

Strategy (K-sharding of the dominant final_w matmul + token-sharded tail):
  - final_w [9216, 19968] contracts over xe features k=(l, h). Core c owns
    encoder positions l in [26c, 26c+26), i.e. contraction rows
    [2496c, 2496c+2496). Weights stream as fp8-e3m4 (x128 scale) packed
    group-major ([18, 128, 20, 512]): one 1.3MB DMA per 512-col output
    group covering all 20 k-tiles, freed right after its 20 matmuls.
    xe is repacked on-chip into [128, 8] bf16 stationary tiles so the PE
    streams 128 contraction rows per cycle.
  - The same position-sharding splits the encoder (attention/LN/FFN) 8x.
  - Each core computes a partial xf [8, 9216]; a ReduceScatter hands core c
    the summed xf for batch c (96 tokens). The TTT tail (neural-memory
    grad step + re-retrieve) runs token-sharded: partial grads are
    AllReduced (small), the final gated output is AllGathered.
  - All non-stream matmuls run float32r (fp22 reads, 4x PE rate vs f32):
    every tensor feeding a matmul is declared float32r natively (bit
    layout identical to f32); PSUM accumulators, biases and collective
    DRAM buffers stay f32; PE-transpose inputs are bitcast down to f32.
  - Weight DMAs ride the scalar(ACT) HWDGE ring, gated behind the arrival
    of x so input/param DMAs on the sync(SP) ring are not starved.
Activations are feature-major [feat, token]; per-token reductions
(l2norm/LN) use ones-matmul partition sums + ones-outer broadcasts.
Encoder token order is (l, b) so xe k-tiles are contiguous slices.
"""

import math

import numpy as np
import ml_dtypes

import concourse.bass as bass
import concourse.mybir as mybir
import concourse.tile as tile
from concourse import bacc
from concourse import bass_utils
from concourse.bass import ds
from concourse.masks import make_identity

F32 = mybir.dt.float32
F32R = mybir.dt.float32r
BF16 = mybir.dt.bfloat16
FP8 = mybir.dt.float8e3
AF = mybir.ActivationFunctionType
OP = mybir.AluOpType

B, S, H, PM, FF, NH = 8, 96, 96, 16, 2048, 2
ALPHA, THETA = 0.999, 0.3
L = PM + 2 * S            # 208 encoder tokens per batch
NC = 8
LSH = L // NC             # 26 positions per core
DK = LSH * H              # 2496 contraction rows per core
DOUT = S * H              # 9216
TQ = B * S                # 768 query-path tokens
HD = H // NH              # 48
NTOK = B * L              # 1664
TSH = B * LSH             # 208 sharded tokens per core
CH = TQ // 2              # 384 (q-path psum chunk)
WSCALE = 128.0            # fp8 weight scale
NKT = 20                  # 128-row k-tiles (2560 = 2496 + 64 pad)
NG = DOUT // 512          # 18 output groups of 512 columns

CFG = {"wbufs": 8}

_CACHE = {}


def _mm(nc, out, lhsT, rhs, start, stop):
    nc.tensor.matmul(out, lhsT, rhs, start=start, stop=stop)


def build(cfg):
    nc = bacc.Bacc("TRN2", target_bir_lowering=False, debug=False, num_devices=NC)

    def din(name, shape, dt=F32):
        return nc.dram_tensor(name, shape, dt, kind="ExternalInput")

    dd = dict(
        xT_d=din("xT", [H, TQ], F32R),
        pmT_d=din("pmT", [H, PM], F32R),
        qwT_d=din("qwT", [H, H], F32R),
        qb_d=din("qb", [H, 1]),
        ipqT_d=din("ipqT", [H, NH, HD], F32R),  # per-head q proj (pre-scaled)
        ipkT_d=din("ipkT", [H, NH, HD], F32R),
        ipvT_d=din("ipvT", [H, H], F32R),
        ipqb_d=din("ipqb", [HD, NH, 1]),
        ipkb_d=din("ipkb", [HD, NH, 1]),
        ipvb_d=din("ipvb", [1, H]),
        opT_d=din("opT", [HD, NH, H], F32R),   # out_proj.T split by head k-tiles
        opb_d=din("opb", [H, 1]),
        ln1w_d=din("ln1w", [H, 1]), ln1b_d=din("ln1b", [H, 1]),
        ln2w_d=din("ln2w", [H, 1]), ln2b_d=din("ln2b", [H, 1]),
        f1T_d=din("f1T", [H, FF], F32R),
        f1b_d=din("f1b", [128, FF // 128, 1]),
        f2T_d=din("f2T", [128, FF // 128, H], F32R),
        f2b_d=din("f2b", [H, 1]),
        kwT_d=din("kwT", [H, H], F32R), kb_d=din("kb", [H, 1]),
        vwT_d=din("vwT", [H, H], F32R), vb_d=din("vb", [H, 1]),
        m1T_d=din("m1T", [H, 2 * H], F32R),
        m1b_d=din("m1b", [H, 2, 1]),
        m2T_d=din("m2T", [H, 2, H], F32R),     # m2_w.T k-tiles
        m2b_d=din("m2b", [H, 1]),
        m2w_d=din("m2w", [H, 2 * H], F32R),
        fbS_d=din("fbS", [S, H]),              # final_b token-major, one batch
        wt_d=din("WTc", [NG, 128, NKT, 512], FP8),
    )
    dd["out_d"] = nc.dram_tensor("outf", [TQ, H], F32, kind="ExternalOutput")

    with tile.TileContext(nc) as tc:
        _body(nc, tc, dd, cfg)
    nc.compile()
    return nc


def _body(nc, tc, dd, cfg):
    from contextlib import ExitStack
    stack = ExitStack()

    def pool(name, bufs, space="SBUF"):
        return stack.enter_context(tc.tile_pool(name=name, bufs=bufs, space=space))

    const = pool("const", 1)
    big = pool("big", 1)
    work = pool("work", 1)
    wstr = pool("wstr", cfg["wbufs"])
    pss = pool("pss", 6, "PSUM")
    psb = pool("psb", 2, "PSUM")
    dram = pool("dram", 1, "DRAM")

    # ---- input x first on the sync ring; everything waits on it least ----
    xT = big.tile([H, TQ], F32R, tag="xT", name="xT")
    nc.sync.dma_start(xT[:], dd["xT_d"][:])
    tok = const.tile([1, 1], F32R, tag="tok", name="tok")

    def ld(dram_t, tag):
        t = const.tile(list(dram_t.shape), dram_t.dtype, tag=tag, name=tag)
        nc.sync.dma_start(t[:], dram_t[:])
        return t

    # params in need-order: q-path first, FFN weights later, tail last
    qwT = ld(dd["qwT_d"], "qwT"); qb = ld(dd["qb_d"], "qb")
    m1T = ld(dd["m1T_d"], "m1T"); m1b = ld(dd["m1b_d"], "m1b")
    m2T = ld(dd["m2T_d"], "m2T"); m2b = ld(dd["m2b_d"], "m2b")
    pmT = ld(dd["pmT_d"], "pmT")
    ipqT = ld(dd["ipqT_d"], "ipqT"); ipkT = ld(dd["ipkT_d"], "ipkT")
    ipvT = ld(dd["ipvT_d"], "ipvT")
    ipqb = ld(dd["ipqb_d"], "ipqb"); ipkb = ld(dd["ipkb_d"], "ipkb")
    opT = ld(dd["opT_d"], "opT"); opb = ld(dd["opb_d"], "opb")
    ln1w = ld(dd["ln1w_d"], "ln1w"); ln1b = ld(dd["ln1b_d"], "ln1b")
    f1T = ld(dd["f1T_d"], "f1T"); f1b = ld(dd["f1b_d"], "f1b")
    f2T = ld(dd["f2T_d"], "f2T"); f2b = ld(dd["f2b_d"], "f2b")
    ln2w = ld(dd["ln2w_d"], "ln2w"); ln2b = ld(dd["ln2b_d"], "ln2b")
    kwT = ld(dd["kwT_d"], "kwT"); kb = ld(dd["kb_d"], "kb")
    vwT = ld(dd["vwT_d"], "vwT"); vb = ld(dd["vb_d"], "vb")
    m2w = ld(dd["m2w_d"], "m2w")

    vb_bc = const.tile([128, H], F32, tag="vb_bc", name="vb_bc")
    nc.sync.dma_start(vb_bc[:], dd["ipvb_d"][:].to_broadcast([128, H]))

    ident = const.tile([128, 128], F32, tag="ident", name="ident")
    make_identity(nc, ident[:])
    ones_f = const.tile([H, 1], F32, tag="ones_f", name="ones_f")
    nc.vector.memset(ones_f[:], 1.0)
    ones_col = const.tile([H, 1], F32R, tag="ones_col", name="ones_col")
    nc.vector.tensor_copy(ones_col[:], ones_f[:])
    ones_rf = const.tile([1, H], F32, tag="ones_rf", name="ones_rf")
    nc.vector.memset(ones_rf[:], 1.0)
    ones_row = const.tile([1, H], F32R, tag="ones_row", name="ones_row")
    nc.vector.tensor_copy(ones_row[:], ones_rf[:])
    zb = const.tile([128, 1], F32, tag="zb", name="zb")
    nc.vector.memset(zb[:], 0.0)
    eps1 = const.tile([1, 1], F32, tag="eps1", name="eps1")
    nc.vector.memset(eps1[:], 1e-5)

    # warm up the collectives firmware (~11us first-use ramp) while the
    # front runs; the real collectives then start in ~1us.
    warm_in = dram.tile([1, 1], F32, tag="warm_in", name="warm_in")
    warm_out = dram.tile([NC, 1], F32, tag="warm_out", name="warm_out")
    nc.sync.dma_start(warm_in[:], zb[0:1, 0:1])
    nc.gpsimd.collective_compute(
        "AllGather", OP.bypass,
        replica_groups=[list(range(NC))],
        ins=[warm_in[:].opt()],
        outs=[warm_out[:].opt()],
    )

    # ---- fp8 weight stream: gpsimd(SWDGE) ring, gated behind the LAST
    # param DMA (vb_bc). Starting the 12MB prefetch only after all small
    # input DMAs complete keeps every front op's DMA-lane semaphore
    # threshold free of weight-DMA increments (no false waits), and the
    # params land at full rate. gpsimd issues ONLY weight DMAs, so its
    # sequencer stalling on ring depth never blocks compute engines.
    nc.vector.tensor_copy(tok[:], vb_bc[0:1, 0:1])
    wbufs = {}
    nwb = cfg["wbufs"]
    for g in range(nwb):
        wb = wstr.tile([128, NKT, 512], FP8, tag="wt", name="wt")
        if g == 0:
            nc.vector.tensor_copy(wb[0:1, 0:1, 0:1], tok[:])
        nc.gpsimd.dma_start(wb[:], dd["wt_d"][g])
        wbufs[g] = wb


    pid = nc.partition_id()
    qoff = pid * LSH

    # ============ F0: shared front (replicated) ============
    # queries -> neural-memory retrieve -> nmm (768 tokens, wide ops,
    # psum-chunked by 384)
    def mm_wide(dst_ap, lhsT_ap, rhs_ap, T, post):
        for c in range((T + CH - 1) // CH):
            sl = slice(c * CH, min((c + 1) * CH, T))
            w = sl.stop - sl.start
            ps = pss.tile([lhsT_ap.shape[-1], CH], F32, tag="ps", name="ps_w")
            _mm(nc, ps[:, :w], lhsT_ap, rhs_ap[:, sl], True, True)
            post(dst_ap[:, sl], ps[:, :w])

    q1w = big.tile([H, TQ], F32R, tag="q1w", name="q1w")
    mm_wide(q1w[:], qwT[:], xT[:], TQ,
            lambda d, s: nc.vector.tensor_scalar_add(d, s, qb[:]))

    # l2norm (wide): inv = min(1/sqrt(sum(q1^2)), 1e12); qry = silu(q1*bc(inv))
    sqw = big.tile([H, TQ], F32R, tag="sqw", name="sqw")
    nc.vector.tensor_mul(sqw[:], q1w[:], q1w[:])
    rsw = work.tile([1, TQ], F32, tag="rsw", name="rsw")
    for c in range(2):
        sl = slice(c * CH, (c + 1) * CH)
        ps = pss.tile([1, CH], F32, tag="ps", name="ps_l2")
        _mm(nc, ps[:], ones_col[:], sqw[:, sl], True, True)
        nc.scalar.activation(rsw[:, sl], ps[:], AF.Sqrt, bias=zb[:1, :])
    nc.vector.tensor_scalar_max(rsw[:], rsw[:], 1e-12)
    inv_f = work.tile([1, TQ], F32, tag="inv_f", name="inv_f")
    nc.vector.reciprocal_approx_fast(inv_f[:], rsw[:])
    invr = work.tile([1, TQ], F32R, tag="invr", name="invr")
    nc.vector.tensor_scalar_min(invr[:], inv_f[:], 1e12)
    qry = big.tile([H, TQ], F32R, tag="sqw", name="qry")  # alias over sqw
    for c in range(2):
        sl = slice(c * CH, (c + 1) * CH)
        psb_ = pss.tile([H, CH], F32, tag="ps", name="ps_l2b")
        _mm(nc, psb_[:], ones_row[:], invr[:, sl], True, True)
        nc.vector.tensor_mul(qry[:, sl], q1w[:, sl], psb_[:])
    qrys = big.tile([H, TQ], F32R, tag="q1w", name="qrys")  # alias over q1w
    nc.scalar.activation(qrys[:], qry[:], AF.Silu, bias=zb[:H, :])

    h1 = [big.tile([H, TQ], F32R, tag=f"h1_{m}", name=f"h1_{m}") for m in range(2)]
    for m in range(2):
        mm_wide(h1[m][:], m1T[:, m * H:(m + 1) * H], qrys[:], TQ,
                lambda d, s, m=m: nc.scalar.activation(
                    d, s, AF.Silu, bias=m1b[:, m, :]))

    xcf = big.tile([H, B, L], F32R, tag="xcf", name="xcf")
    nc.vector.tensor_copy(xcf[:, :, 0:PM],
                          pmT[:].unsqueeze(1).to_broadcast([H, B, PM]))
    nc.vector.tensor_copy(xcf[:, :, PM + S:L],
                          xT[:].rearrange("h (b s) -> h b s", b=B))
    for c in range(2):
        sl = slice(c * CH, (c + 1) * CH)
        ps = pss.tile([H, CH], F32, tag="ps", name="ps_nmm")
        _mm(nc, ps[:], m2T[:, 0, :], h1[0][:, sl], True, False)
        _mm(nc, ps[:], m2T[:, 1, :], h1[1][:, sl], False, True)
        nc.vector.tensor_scalar_add(
            xcf[:, 4 * c:4 * (c + 1), PM:PM + S],
            ps[:].rearrange("h (b s) -> h b s", b=4), m2b[:])

    # k projection (all tokens) + q projection (only my 26 positions/batch)
    xcf_flat = xcf[:].rearrange("h b l -> h (b l)")
    kf = big.tile([HD, NH, B, L], F32R, tag="kf", name="kf")
    ECH = NTOK // 4
    for hh in range(NH):
        for c in range(4):
            sl = slice(c * ECH, (c + 1) * ECH)
            ps = pss.tile([HD, ECH], F32, tag="ps", name="ps_k")
            _mm(nc, ps[:], ipkT[:, hh, :], xcf_flat[:, sl], True, True)
            nc.vector.tensor_scalar_add(
                kf[:].rearrange("d n b l -> d n (b l)")[:, hh, sl],
                ps[:], ipkb[:, hh, :])
    q_sel = big.tile([HD, NH, B, LSH], F32R, tag="q_sel", name="q_sel")
    for hh in range(NH):
        ps = pss.tile([HD, TSH], F32, tag="ps", name="ps_q")
        _mm(nc, ps[:], ipqT[:, hh, :], xcf[:, :, ds(qoff, LSH)], True, True)
        nc.vector.tensor_scalar_add(
            q_sel[:, hh, :, :].rearrange("d b l -> d (b l)"),
            ps[:], ipqb[:, hh, :])

    # v token-major per batch: [128+80, B, H]
    v_tm0 = big.tile([128, B, H], F32R, tag="v_tm0", name="v_tm0")
    v_tm1 = big.tile([80, B, H], F32R, tag="v_tm1", name="v_tm1")
    for b in range(B):
        for tt, dst, npart in ((0, v_tm0, 128), (1, v_tm1, 80)):
            ps = pss.tile([128, H], F32, tag="ps", name="ps_v")
            toks = slice(b * L + tt * 128, b * L + tt * 128 + npart)
            _mm(nc, ps[:npart, :], xcf_flat[:, toks], ipvT[:], True, True)
            nc.vector.tensor_add(dst[:, b, :], ps[:npart, :], vb_bc[:npart, :])

    # ============ F1: attention, 3 (b,hh)-pairs per psum tile ============
    # (matmul output partition base must be 0/32/64)
    of = big.tile([HD, NH, B, LSH], F32R, tag="of", name="of")
    pairs = [(b, hh) for b in range(B) for hh in range(NH)]
    for ti in range(6):
        grp = pairs[ti * 3:min((ti + 1) * 3, len(pairs))]
        es4 = work.tile([H, L], F32, tag="es4", name="es4", bufs=3)
        nc.vector.memset(es4[:], 0.0)
        for j, (b, hh) in enumerate(grp):
            ps_s = pss.tile([LSH, L], F32, tag="ps", name="ps_s")
            _mm(nc, ps_s[:], q_sel[:, hh, b, :], kf[:, hh, b, :], True, True)
            nc.vector.tensor_copy(es4[32 * j:32 * j + LSH, :], ps_s[:])
        e4 = work.tile([H, L], F32, tag="e4", name="e4", bufs=3)
        den = work.tile([H, 1], F32, tag="den", name="den", bufs=3)
        nc.scalar.activation(e4[:], es4[:], AF.Exp, bias=zb[:H, :],
                             accum_out=den[:])
        rden = work.tile([H, 1], F32, tag="rden", name="rden", bufs=3)
        nc.vector.reciprocal(rden[:], den[:])
        a4 = work.tile([H, L], F32, tag="a4", name="a4", bufs=3)
        nc.vector.tensor_scalar_mul(a4[:], e4[:], rden[:])
        ats = []
        for tt, npart in ((0, 128), (1, 80)):
            ps_t = pss.tile([128, H], F32, tag="ps", name="ps_t")
            nc.tensor.transpose(ps_t[:npart, :],
                                a4[:, tt * 128:tt * 128 + npart], ident[:H, :H])
            at = work.tile([128, H], F32R, tag=f"at{tt}", name=f"at{tt}", bufs=3)
            nc.vector.tensor_copy(at[:npart, :], ps_t[:npart, :])
            ats.append((at, npart))
        for j, (b, hh) in enumerate(grp):
            ps_o = pss.tile([HD, LSH], F32, tag="ps", name="ps_o")
            for tt, (at, npart) in enumerate(ats):
                _mm(nc, ps_o[:], v_tm0[:, b, hh * HD:(hh + 1) * HD] if tt == 0
                    else v_tm1[:, b, hh * HD:(hh + 1) * HD],
                    at[:npart, 32 * j:32 * j + LSH], tt == 0, tt == 1)
            nc.vector.tensor_copy(of[:, hh, b, :], ps_o[:])

    # ============ encoder tail on my 208 tokens, (l b) order ============
    ps_op = pss.tile([H, TSH], F32, tag="ps", name="ps_op")
    for hh in range(NH):
        _mm(nc, ps_op[:].rearrange("h (l b) -> h l b", l=LSH),
            opT[:, hh, :],
            of[:, hh, :, :].rearrange("d b l -> d l b"), hh == 0, hh == 1)
    x1 = big.tile([H, TSH], F32R, tag="x1", name="x1")
    tmp_op = work.tile([H, TSH], F32, tag="w208", name="tmp_op")
    nc.vector.tensor_scalar_add(tmp_op[:], ps_op[:], opb[:])
    nc.vector.tensor_add(x1[:].rearrange("h (l b) -> h l b", l=LSH),
                         tmp_op[:].rearrange("h (l b) -> h l b", l=LSH),
                         xcf[:, :, ds(qoff, LSH)].rearrange("h b l -> h l b"))

    def ln_wide(src_ap, dst_ap, w_ap, b_ap):
        T = src_ap.shape[-1]
        sq = work.tile([H, TSH], F32R, tag="ln_sq", name="ln_sq")
        nc.vector.tensor_mul(sq[:, :T], src_ap, src_ap)
        ps_s = pss.tile([1, TSH], F32, tag="ps", name="ps_lns")
        _mm(nc, ps_s[:1, :T], ones_col[:], src_ap, True, True)
        ps_q = pss.tile([1, TSH], F32, tag="ps", name="ps_lnq")
        _mm(nc, ps_q[:1, :T], ones_col[:], sq[:, :T], True, True)
        mean = work.tile([1, TSH], F32R, tag="ln_mean", name="ln_mean")
        nc.vector.tensor_scalar_mul(mean[:1, :T], ps_s[:1, :T], 1.0 / H)
        var = work.tile([1, TSH], F32, tag="ln_var", name="ln_var")
        nc.vector.tensor_scalar_mul(var[:1, :T], ps_q[:1, :T], 1.0 / H)
        m2t = work.tile([1, TSH], F32, tag="ln_m2", name="ln_m2")
        nc.vector.tensor_mul(m2t[:1, :T], mean[:1, :T], mean[:1, :T])
        nc.vector.tensor_sub(var[:1, :T], var[:1, :T], m2t[:1, :T])
        sd = work.tile([1, TSH], F32, tag="ln_sd", name="ln_sd")
        nc.scalar.activation(sd[:1, :T], var[:1, :T], AF.Sqrt, bias=eps1[:])
        rstd_f = work.tile([1, TSH], F32, tag="ln_rstdf", name="ln_rstdf")
        nc.vector.reciprocal_approx_fast(rstd_f[:1, :T], sd[:1, :T])
        rstd = work.tile([1, TSH], F32R, tag="ln_rstd", name="ln_rstd")
        nc.vector.tensor_copy(rstd[:1, :T], rstd_f[:1, :T])
        ps_a = pss.tile([H, TSH], F32, tag="ps", name="ps_lna")
        _mm(nc, ps_a[:, :T], ones_row[:], rstd[:1, :T], True, True)
        ps_c = pss.tile([H, TSH], F32, tag="ps", name="ps_lnc")
        _mm(nc, ps_c[:, :T], ones_row[:], mean[:1, :T], True, True)
        t1 = work.tile([H, TSH], F32, tag="ln_t1", name="ln_t1")
        nc.vector.tensor_sub(t1[:, :T], src_ap, ps_c[:, :T])
        nc.vector.tensor_mul(t1[:, :T], t1[:, :T], ps_a[:, :T])
        nc.vector.tensor_scalar(dst_ap, t1[:, :T], w_ap, b_ap, OP.mult, OP.add)

    x1n = big.tile([H, TSH], F32R, tag="x1n", name="x1n")
    ln_wide(x1[:], x1n[:], ln1w[:], ln1b[:])

    ps_f2 = pss.tile([H, TSH], F32, tag="ps", name="ps_f2")
    for m in range(FF // 128):
        psf = pss.tile([128, TSH], F32, tag="ps", name="ps_f1")
        _mm(nc, psf[:], f1T[:, m * 128:(m + 1) * 128], x1n[:], True, True)
        h_ffn = work.tile([128, TSH], F32R, tag="h_ffn", name="h_ffn", bufs=3)
        nc.scalar.activation(h_ffn[:], psf[:], AF.Silu, bias=f1b[:, m, :])
        _mm(nc, ps_f2[:], f2T[:, m, :], h_ffn[:], m == 0, m == FF // 128 - 1)
    x2 = big.tile([H, TSH], F32R, tag="x2", name="x2")
    tmp_ff = work.tile([H, TSH], F32, tag="w208", name="tmp_ff")
    nc.vector.tensor_scalar_add(tmp_ff[:], ps_f2[:], f2b[:])
    nc.vector.tensor_add(x2[:], tmp_ff[:], x1n[:])

    e2 = big.tile([H, TSH], F32R, tag="x1", name="e2")  # alias over x1
    ln_wide(x2[:], e2[:], ln2w[:], ln2b[:])
    xef = big.tile([H, TSH], F32R, tag="x2", name="xef")  # alias over x2
    nc.scalar.activation(xef[:], e2[:], AF.Silu, bias=zb[:H, :])

    # repack xe into bf16 k-tiles [128, B]: row 96*ll+h of tile kt=(96*ll)//128
    xeT = [big.tile([128, B], BF16, tag=f"xeT{kt}", name=f"xeT{kt}")
           for kt in range(NKT)]
    nc.vector.memset(xeT[NKT - 1][64:128, :], 0.0)
    # 32-row chunks: DVE partition ranges must be pow2-aligned at their base
    for ll in range(LSH):
        cols = xef[:, 8 * ll:8 * ll + 8]
        for c3 in range(3):
            r = 96 * ll + 32 * c3
            kt, off = r // 128, r % 128
            nc.vector.tensor_copy(xeT[kt][off:off + 32, :],
                                  cols[32 * c3:32 * c3 + 32, :])

    # ============ F2: big matmul (fp8 stream, K=128) ============
    # three ReduceScatter chunks (groups 0-8 / 9-14 / 15-17, all
    # token-aligned): the early ones drain under the remaining matmuls,
    # only the last small chunk (48KB wire, 16 tokens) stays exposed.
    RSG = [(0, 9), (9, 6), (15, 3)]
    ar_h = [dram.tile([B, n * 512], F32, tag=f"ar_in{h}", name=f"ar_in{h}")
            for h, (_, n) in enumerate(RSG)]
    rs_h = [dram.tile([n * 512 // H, H], F32, tag=f"rs_out{h}",
                      name=f"rs_out{h}")
            for h, (_, n) in enumerate(RSG)]
    rs_tok = [g0 * 512 // H for g0, _ in RSG] + [S]
    for g in range(NG):
        if g + nwb < NG:
            wb = wstr.tile([128, NKT, 512], FP8, tag="wt", name="wt")
            nc.gpsimd.dma_start(wb[:], dd["wt_d"][g + nwb])
            wbufs[g + nwb] = wb
        psx = psb.tile([B, 512], F32, tag="psx", name="psx")
        for kt in range(NKT):
            nc.tensor.matmul(psx[:], xeT[kt][:], wbufs[g][:, kt, :],
                             start=(kt == 0), stop=(kt == NKT - 1))
        xfp = work.tile([B, 512], F32, tag="xfp", name="xfp", bufs=3)
        nc.vector.tensor_scalar_mul(xfp[:], psx[:], 1.0 / WSCALE)
        h = next(i for i, (g0, n) in enumerate(RSG) if g0 <= g < g0 + n)
        nc.sync.dma_start(
            ar_h[h][:, 512 * (g - RSG[h][0]):512 * (g - RSG[h][0]) + 512],
            xfp[:])
        if g == RSG[h][0] + RSG[h][1] - 1:
            nc.gpsimd.collective_compute(
                "ReduceScatter", OP.add,
                replica_groups=[list(range(NC))],
                ins=[ar_h[h][:].opt()],
                outs=[rs_h[h][:].opt()],
            )

    # ============ T: tail (token-sharded: this core owns batch pid) ============
    # forward runs per RS chunk (48/32/16 tokens) so the early chunks hide
    # under the stream tail and the later RS chunks. Token-major tiles are
    # [128, 96] with chunk rows at bases 0/64/96 (pads zeroed for the K=128
    # grad contraction).
    ROWB = [0, 64, 96]
    tm_names = ["kp_tm", "h_tm0", "h_tm1", "dpred_tm", "dz_tm0", "dz_tm1"]
    zrs_f = const.tile([32, S], F32, tag="zrs_f", name="zrs_f")
    nc.vector.memset(zrs_f[:], 0.0)
    zrs = const.tile([32, S], F32R, tag="zrs", name="zrs")
    nc.vector.tensor_copy(zrs[:], zrs_f[:])
    tms = {}
    for nm in tm_names:
        t = big.tile([128, S], F32R, tag=nm, name=nm)
        nc.vector.tensor_copy(t[32:64, :], zrs[:])
        nc.vector.tensor_copy(t[96:128, :], zrs[:])
        tms[nm] = t
    fb_pad = const.tile([S // 2, 3, H], F32, tag="fb_pad", name="fb_pad")
    for c in range(3):
        nc.sync.dma_start(fb_pad[0:rs_tok[c + 1] - rs_tok[c], c, :],
                          dd["fbS_d"][rs_tok[c]:rs_tok[c + 1], :])

    def tm_store(src_ap, nm, c, n):
        """transpose [H, n] chunk -> token-major rows of tms[nm]."""
        ps_t = pss.tile([S, S], F32, tag="ps", name=f"ps_{nm}")
        nc.tensor.transpose(ps_t[:n, :], src_ap.bitcast(F32), ident[:H, :H])
        nc.vector.tensor_copy(tms[nm][ROWB[c]:ROWB[c] + n, :], ps_t[:n, :])

    xff = big.tile([H, S], F32R, tag="xff", name="xff")
    kp = big.tile([H, S], F32R, tag="kp", name="kp")
    vp = big.tile([H, S], F32R, tag="vp", name="vp")
    hs = [big.tile([H, S], F32R, tag=f"h_{m}", name=f"h_{m}") for m in range(2)]
    sp = [work.tile([H, S], F32, tag=f"sp_{m}", name=f"sp_{m}") for m in range(2)]
    dpr = big.tile([H, S], F32R, tag="dpr", name="dpr")
    dzs = [work.tile([H, S], F32R, tag=f"dz_{m}", name=f"dz_{m}")
           for m in range(2)]

    for c in range(3):
        t0, t1 = rs_tok[c], rs_tok[c + 1]
        n = t1 - t0
        ts = slice(t0, t1)
        xfc = work.tile([S, H], F32, tag=f"xfc{c}", name=f"xfc{c}")
        nc.sync.dma_start(xfc[:n, :], rs_h[c][:])
        nc.vector.tensor_add(xfc[:n, :], xfc[:n, :], fb_pad[0:n, c, :])
        ps_x = pss.tile([H, S], F32, tag="ps", name="ps_xff")
        nc.tensor.transpose(ps_x[:, :n], xfc[:n, :], ident[:n, :n])
        nc.vector.tensor_copy(xff[:, ts], ps_x[:, :n])
        ps_k = pss.tile([H, S], F32, tag="ps", name="ps_kp")
        _mm(nc, ps_k[:, :n], kwT[:], xff[:, ts], True, True)
        nc.vector.tensor_scalar_add(kp[:, ts], ps_k[:, :n], kb[:])
        ps_v = pss.tile([H, S], F32, tag="ps", name="ps_vp")
        _mm(nc, ps_v[:, :n], vwT[:], xff[:, ts], True, True)
        nc.vector.tensor_scalar_add(vp[:, ts], ps_v[:, :n], vb[:])
        tm_store(kp[:, ts], "kp_tm", c, n)
        for m in range(2):
            ps_z = pss.tile([H, S], F32, tag="ps", name="ps_z")
            _mm(nc, ps_z[:, :n], m1T[:, m * H:(m + 1) * H], kp[:, ts],
                True, True)
            z_m = work.tile([H, S], F32, tag=f"z_{m}", name=f"z_{m}", bufs=2)
            nc.vector.tensor_scalar_add(z_m[:, :n], ps_z[:, :n], m1b[:, m, :])
            sg_m = work.tile([H, S], F32, tag=f"sg_{m}", name=f"sg_{m}", bufs=2)
            nc.scalar.activation(sg_m[:, :n], z_m[:, :n], AF.Sigmoid,
                                 bias=zb[:H, :])
            nc.vector.tensor_mul(hs[m][:, ts], z_m[:, :n], sg_m[:, :n])
            t1m = work.tile([H, S], F32, tag="t1_m", name="t1_m", bufs=2)
            nc.vector.tensor_sub(t1m[:, :n], z_m[:, :n], hs[m][:, ts])
            nc.vector.tensor_scalar_add(t1m[:, :n], t1m[:, :n], 1.0)
            nc.vector.tensor_mul(sp[m][:, ts], sg_m[:, :n], t1m[:, :n])
            tm_store(hs[m][:, ts], f"h_tm{m}", c, n)
        ps_p = pss.tile([H, S], F32, tag="ps", name="ps_p")
        _mm(nc, ps_p[:, :n], m2T[:, 0, :], hs[0][:, ts], True, False)
        _mm(nc, ps_p[:, :n], m2T[:, 1, :], hs[1][:, ts], False, True)
        pr = work.tile([H, S], F32, tag="pr", name="pr", bufs=2)
        nc.vector.tensor_scalar_add(pr[:, :n], ps_p[:, :n], m2b[:])
        nc.vector.tensor_sub(dpr[:, ts], pr[:, :n], vp[:, ts])
        nc.vector.tensor_scalar_mul(dpr[:, ts], dpr[:, ts], 2.0 / (TQ * H))
        tm_store(dpr[:, ts], "dpred_tm", c, n)
        for m in range(2):
            ps_dh = pss.tile([H, S], F32, tag="ps", name="ps_dh")
            _mm(nc, ps_dh[:, :n], m2w[:, m * H:(m + 1) * H], dpr[:, ts],
                True, True)
            nc.vector.tensor_mul(dzs[m][:, ts], ps_dh[:, :n], sp[m][:, ts])
            tm_store(dzs[m][:, ts], f"dz_tm{m}", c, n)

    kp_tm, dpred_tm = tms["kp_tm"], tms["dpred_tm"]
    h_tm = [tms["h_tm0"], tms["h_tm1"]]
    dz_tm = [tms["dz_tm0"], tms["dz_tm1"]]
    NGR = 4 * H + 3  # grad pack: g1_0 g1_1 g2_0 g2_1 gb1_0 gb1_1 gb2
    gpack = big.tile([H, NGR], F32, tag="gpack", name="gpack")
    nc.vector.reduce_sum(gpack[:, 4 * H + 2:4 * H + 3], dpr[:],
                         axis=mybir.AxisListType.X)
    for m in range(2):
        nc.vector.reduce_sum(gpack[:, 4 * H + m:4 * H + m + 1], dzs[m][:],
                             axis=mybir.AxisListType.X)

    for m in range(2):
        ps_g = pss.tile([H, H], F32, tag="ps", name="ps_g1")
        _mm(nc, ps_g[:], kp_tm[:], dz_tm[m][:], True, True)
        nc.vector.tensor_copy(gpack[:, m * H:(m + 1) * H], ps_g[:])
        ps_g2 = pss.tile([H, H], F32, tag="ps", name="ps_g2")
        _mm(nc, ps_g2[:], h_tm[m][:], dpred_tm[:], True, True)
        nc.vector.tensor_copy(gpack[:, (2 + m) * H:(3 + m) * H], ps_g2[:])

    g_in = dram.tile([H, NGR], F32, tag="g_in", name="g_in")
    g_out = dram.tile([H, NGR], F32, tag="g_out", name="g_out")
    nc.sync.dma_start(g_in[:], gpack[:])
    nc.gpsimd.collective_compute(
        "AllReduce", OP.add,
        replica_groups=[list(range(NC))],
        ins=[g_in[:].opt()],
        outs=[g_out[:].opt()],
    )
    gsum = big.tile([H, NGR], F32, tag="gpack", name="gsum")  # alias
    nc.sync.dma_start(gsum[:], g_out[:])
    gth = work.tile([H, NGR], F32, tag="gth", name="gth")
    nc.vector.tensor_scalar_mul(gth[:], gsum[:], THETA)

    # retrieve query path overlaps the grad AllReduce (depends only on xff)
    ps_q2 = pss.tile([H, S], F32, tag="ps", name="ps_q2")
    _mm(nc, ps_q2[:], qwT[:], xff[:], True, True)
    q2r = work.tile([H, S], F32R, tag="q2r", name="q2r")
    nc.vector.tensor_scalar_add(q2r[:], ps_q2[:], qb[:])
    sq2 = work.tile([H, S], F32R, tag="sq2", name="sq2")
    nc.vector.tensor_mul(sq2[:], q2r[:], q2r[:])
    ps_ss = pss.tile([1, S], F32, tag="ps", name="ps_ss")
    _mm(nc, ps_ss[:], ones_col[:], sq2[:], True, True)
    rs2 = work.tile([1, S], F32, tag="rs2", name="rs2")
    nc.scalar.activation(rs2[:], ps_ss[:], AF.Sqrt, bias=zb[:1, :])
    nc.vector.tensor_scalar_max(rs2[:], rs2[:], 1e-12)
    inv2f = work.tile([1, S], F32, tag="inv2f", name="inv2f")
    nc.vector.reciprocal_approx_fast(inv2f[:], rs2[:])
    inv2 = work.tile([1, S], F32R, tag="inv2", name="inv2")
    nc.vector.tensor_scalar_min(inv2[:], inv2f[:], 1e12)
    ps_bc = pss.tile([H, S], F32, tag="ps", name="ps_bc")
    _mm(nc, ps_bc[:], ones_row[:], inv2[:], True, True)
    q2 = work.tile([H, S], F32R, tag="q2", name="q2")
    nc.vector.tensor_mul(q2[:], q2r[:], ps_bc[:])

    nm1T = big.tile([H, 2 * H], F32R, tag="nm1T", name="nm1T")
    nm1b = work.tile([H, 2], F32, tag="nm1b", name="nm1b")
    nm2T = big.tile([H, 2, H], F32R, tag="nm2T", name="nm2T")
    nm2b = work.tile([H, 1], F32, tag="nm2b", name="nm2b")
    for m in range(2):
        msl = slice(m * H, (m + 1) * H)
        nc.vector.tensor_scalar_mul(nm1T[:, msl], m1T[:, msl], ALPHA)
        nc.vector.tensor_sub(nm1T[:, msl], nm1T[:, msl], gth[:, msl])
        nc.vector.tensor_scalar_mul(nm2T[:, m, :], m2T[:, m, :], ALPHA)
        nc.vector.tensor_sub(nm2T[:, m, :], nm2T[:, m, :],
                             gth[:, (2 + m) * H:(3 + m) * H])
        nc.vector.tensor_scalar_mul(nm1b[:, m:m + 1], m1b[:, m, :], ALPHA)
        nc.vector.tensor_sub(nm1b[:, m:m + 1], nm1b[:, m:m + 1],
                             gth[:, 4 * H + m:4 * H + m + 1])
    nc.vector.tensor_scalar_mul(nm2b[:], m2b[:], ALPHA)
    nc.vector.tensor_sub(nm2b[:], nm2b[:], gth[:, 4 * H + 2:4 * H + 3])

    uu = []
    for m in range(2):
        ps_u = pss.tile([H, S], F32, tag="ps", name="ps_u")
        _mm(nc, ps_u[:], nm1T[:, m * H:(m + 1) * H], q2[:], True, True)
        u_m = work.tile([H, S], F32R, tag=f"u_{m}", name=f"u_{m}")
        nc.scalar.activation(u_m[:], ps_u[:], AF.Silu, bias=nm1b[:, m:m + 1])
        uu.append(u_m)
    ps_y = pss.tile([H, S], F32, tag="ps", name="ps_y")
    _mm(nc, ps_y[:], nm2T[:, 0, :], uu[0][:], True, False)
    _mm(nc, ps_y[:], nm2T[:, 1, :], uu[1][:], False, True)
    sg_y = work.tile([H, S], F32, tag="sg_y", name="sg_y")
    nc.scalar.activation(sg_y[:], ps_y[:], AF.Sigmoid, bias=nm2b[:])
    ot = work.tile([H, S], F32R, tag="ot", name="ot")
    nc.vector.tensor_mul(ot[:], xff[:], sg_y[:])
    ps_ot = pss.tile([S, S], F32, tag="ps", name="ps_ot")
    nc.tensor.transpose(ps_ot[:], ot[:].bitcast(F32), ident[:H, :H])
    ot_tm = big.tile([S, H], F32, tag="ot_tm", name="ot_tm")
    nc.vector.tensor_copy(ot_tm[:], ps_ot[:])

    ag_in = dram.tile([S, H], F32, tag="ag_in", name="ag_in")
    ag_out = dram.tile([TQ, H], F32, tag="ag_out", name="ag_out")
    nc.sync.dma_start(ag_in[:], ot_tm[:])
    nc.gpsimd.collective_compute(
        "AllGather", OP.bypass,
        replica_groups=[list(range(NC))],
        ins=[ag_in[:].opt()],
        outs=[ag_out[:].opt()],
    )
    nc.sync.dma_start(dd["out_d"][:], ag_out[:])

    stack.close()


def prep_inmaps(inputs, cfg=None):
    cfg = cfg or CFG
    f32 = np.float32

    def T(a):
        return np.ascontiguousarray(np.asarray(a, f32).T)

    x = np.asarray(inputs["x"], f32)
    ipw = np.asarray(inputs["in_proj_w"], f32)   # [288, 96]
    ipb = np.asarray(inputs["in_proj_b"], f32)   # [288]
    sc = 1.0 / math.sqrt(HD)
    qw_part = ipw[0:H] * sc
    qb_part = ipb[0:H] * sc
    kw_part = ipw[H:2 * H]
    kb_part = ipb[H:2 * H]
    vw_part = ipw[2 * H:3 * H]
    vb_part = ipb[2 * H:3 * H]

    ipqT = np.ascontiguousarray(qw_part.T.reshape(H, NH, HD))
    ipkT = np.ascontiguousarray(kw_part.T.reshape(H, NH, HD))
    ipqb = np.ascontiguousarray(qb_part.reshape(NH, HD).T.reshape(HD, NH, 1))
    ipkb = np.ascontiguousarray(kb_part.reshape(NH, HD).T.reshape(HD, NH, 1))

    opw = np.asarray(inputs["out_proj_w"], f32)  # [96, 96]
    opT = np.ascontiguousarray(opw.T.reshape(NH, HD, H).transpose(1, 0, 2))

    f1b = np.asarray(inputs["ff1_b"], f32).reshape(FF // 128, 128, 1)
    f1b = np.ascontiguousarray(f1b.transpose(1, 0, 2))
    f2T = T(inputs["ff2_w"])                     # [2048, 96]
    f2T = np.ascontiguousarray(f2T.reshape(FF // 128, 128, H).transpose(1, 0, 2))

    m1b = np.ascontiguousarray(
        np.asarray(inputs["m1_b"], f32).reshape(2, H, 1).transpose(1, 0, 2))
    m2T = np.ascontiguousarray(
        T(inputs["m2_w"]).reshape(2, H, H).transpose(1, 0, 2))  # [96, 2, 96]

    fwT = np.ascontiguousarray(np.asarray(inputs["final_w"], f32).T)
    fbS = np.ascontiguousarray(
        np.asarray(inputs["final_b"], f32).reshape(S, H))

    col = lambda k: np.ascontiguousarray(np.asarray(inputs[k], f32).reshape(-1, 1))
    base = dict(
        xT=T(x.reshape(TQ, H)),
        pmT=T(inputs["persistent_memory"]),
        qwT=T(inputs["q_w"]), qb=col("q_b"),
        ipqT=ipqT, ipkT=ipkT, ipvT=np.ascontiguousarray(vw_part.T),
        ipqb=ipqb, ipkb=ipkb,
        ipvb=np.ascontiguousarray(vb_part.reshape(1, H)),
        opT=opT, opb=col("out_proj_b"),
        ln1w=col("ln1_w"), ln1b=col("ln1_b"),
        ln2w=col("ln2_w"), ln2b=col("ln2_b"),
        f1T=T(inputs["ff1_w"]), f1b=f1b,
        f2T=f2T, f2b=col("ff2_b"),
        kwT=T(inputs["k_w"]), kb=col("k_b"),
        vwT=T(inputs["v_w"]), vb=col("v_b"),
        m1T=T(inputs["m1_w"]), m1b=m1b,
        m2T=m2T, m2b=col("m2_b"),
        m2w=np.ascontiguousarray(np.asarray(inputs["m2_w"], f32)),
        fbS=fbS,
    )
    in_maps = []
    for c in range(NC):
        m = dict(base)
        shard = fwT[c * DK:(c + 1) * DK]                    # [2496, 9216]
        padded = np.zeros((NKT * 128, DOUT), f32)
        padded[:DK] = shard * WSCALE
        q8 = padded.astype(ml_dtypes.float8_e3m4)
        # [NG, 128, NKT, 512]: wt[g, p, kt, j] = q8[128*kt+p, 512*g+j]
        m["WTc"] = np.ascontiguousarray(
            q8.reshape(NKT, 128, NG, 512).transpose(2, 1, 0, 3))
        in_maps.append(m)
    return in_maps


def get_nc(cfg=None):
    cfg = cfg or CFG
    key = tuple(sorted((k, str(v)) for k, v in cfg.items()))
    if key not in _CACHE:
        _CACHE[key] = build(cfg)
    return _CACHE[key]


def kernel(**inputs):
    nc = get_nc()
    in_maps = prep_inmaps(inputs)
    res = bass_utils.run_bass_kernel_spmd(
        nc, in_maps, core_ids=list(range(NC)), trace=False
    )
    outf = res.results[0]["outf"]  # [768, 96] token-major
    return np.ascontiguousarray(outf).reshape(B, S, H)


if __name__ == "__main__":
    print("building...")
    get_nc()
    print("built")
